# revision 32
# baseline (speedup 1.0000x reference)
"""HOPE block kernel for 8 Trainium2 NeuronCores — v3.

Sharding: 8 shards = (batch b in 0..3, sequence half in 0..1), 2048 tokens each.
v3 replaces v2's "rebuild prefix memory locally" phase A with a pairwise
AllGather of the linear-attention memory state M (256KB): core 2b runs the
first half of batch b's sequence and its post-scan M state IS the mid-sequence
memory the partner core 2b+1 needs.  Every core runs its local scan with M0=0;
after the gather, y += q @ M_remote is added (M_remote is scaled by a per-core
0/1 input so even cores add zero; the SPMD program stays uniform).

Other changes vs v2:
 - All weights are re-laid-out on the host so every DMA line is >=2KB
   contiguous (v2's 512B-1KB gather lines made the HBM counter run at ~95%
   during attention).  wq/wk/wv/wo stay SBUF-resident.
 - The scan is feature-major: y^T accumulates straight into the [feat, tok]
   layout the wo projection consumes (no y transposes), and each matmul
   processes a head PAIR via partition-offset tile packing (K=64 / M=64
   matmuls land on disjoint 64-row/64-col PE tile groups and run
   concurrently).
 - The CMS FFN down-projection accumulates over the full 4096 hidden dim in
   one PSUM group (no half-split partial buffer), and the last level emits
   token-major output directly so the final residual add + store needs no
   transposes.
"""
import sys
if '/opt/trn_rl_repo' not in sys.path:
    sys.path.insert(0, '/opt/trn_rl_repo')

from contextlib import ExitStack
import numpy as np


def _ensure_ntff_hook():
    """Register the axon NTFF profile hook when the image's antenv stub
    lacks `axon_hooks` — otherwise BASS_TRACE runs skip tracing and
    exec_time_ns comes back None. No-op when the real module exists."""
    import types, contextlib, ctypes, os
    try:
        from antenv.axon_hooks import get_axon_ntff_profile_hook  # noqa: F401
        return
    except ImportError:
        pass
    hook = None
    so_path = '/opt/axon/libaxon_pjrt.so'
    if os.path.exists(so_path):
        try:
            lib = ctypes.CDLL(so_path)
        except OSError:
            lib = None
        if lib is not None and hasattr(lib, 'axon_start_nrt_profile'):
            lib.axon_start_nrt_profile.argtypes = [
                ctypes.POINTER(ctypes.c_int64), ctypes.c_size_t]
            lib.axon_start_nrt_profile.restype = ctypes.c_int64
            lib.axon_stop_nrt_profile.argtypes = [ctypes.c_char_p]
            lib.axon_stop_nrt_profile.restype = ctypes.c_int64

            @contextlib.contextmanager
            def hook(output_dir, device_ids):
                import jax
                jax.devices()  # force PJRT init so the .so's client exists
                if device_ids:
                    ids = (ctypes.c_int64 * len(device_ids))(*device_ids)
                    rc = lib.axon_start_nrt_profile(ids, len(device_ids))
                else:
                    rc = lib.axon_start_nrt_profile(None, 0)
                if rc != 0:
                    raise RuntimeError(f"axon_start_nrt_profile rc={rc}")
                try:
                    yield
                finally:
                    n = lib.axon_stop_nrt_profile(str(output_dir).encode())
                    print(f"profile: {n} file(s) written to {output_dir}",
                          file=sys.stderr)

    try:
        import antenv
    except ImportError:
        return
    mod = types.ModuleType('antenv.axon_hooks')
    _h = hook
    mod.get_axon_ntff_profile_hook = lambda: _h
    mod.set_axon_ntff_profile_hook = lambda h: None
    sys.modules['antenv.axon_hooks'] = mod
    antenv.axon_hooks = mod


_ensure_ntff_hook()

import concourse.bass as bass
import concourse.tile as tile
from concourse import mybir
from concourse.bass_utils import run_bass_kernel_spmd
from concourse.masks import make_identity

f32 = mybir.dt.float32
bf16 = mybir.dt.bfloat16
f8 = mybir.dt.float8e4
AF = mybir.ActivationFunctionType
ALU = mybir.AluOpType
PM = mybir.MatmulPerfMode

W8_SCALE = 64.0     # host-side multiplier on fp8 FFN weights (clears the
INV_W8 = 1.0 / W8_SCALE  # e4m3 subnormal zone); un-done via activation scale

DIM = 1024
HEADS = 16
HD = 64
B, S = 4, 4096
LEVELS = 3
HID = 4 * DIM
CHUNK = 128
EPS = 1e-5
P = 128

N_CORES = 8
T_OWN = S // 2      # tokens per core
BLK = 512           # token block for the attention phase
D_T = DIM // P      # 8 feature tiles
H_T = HID // P      # 32 hidden tiles
N_BLK = T_OWN // BLK
NTT = BLK // P      # 4 token tiles per block
N_OWN_T = T_OWN // P
TT = 512            # FFN token tile

MAX_WAITS = 1

REPLICA_PAIRS = [[0, 1], [2, 3], [4, 5], [6, 7]]


def _split_multi_waits(nc, max_waits=MAX_WAITS):
    """Walrus in this toolchain encodes at most `max_waits` sem waits per
    instruction; split extra waits onto same-engine NOPs placed just before."""
    for f in nc.m.functions:
        for bb in f.blocks:
            insts = list(bb.instructions)
            if not any(
                i.sync_info and i.sync_info.on_wait and len(i.sync_info.on_wait) > max_waits
                for i in insts
            ):
                continue
            new = []
            for inst in insts:
                si = inst.sync_info
                waits = list(si.on_wait) if si and si.on_wait else []
                if len(waits) > max_waits:
                    head, rest = waits[:-max_waits], waits[-max_waits:]
                    while head:
                        chunk, head = head[:max_waits], head[max_waits:]
                        nop = mybir.InstNoOp(name=nc.get_next_instruction_name(), ins=[], outs=[])
                        nop.engine = inst.engine
                        nop.sync_info = mybir.SyncInfo(on_wait=chunk, on_update=[])
                        nc.register_instruction(nop, overwrite=True)
                        new.append(nop)
                    inst.sync_info = mybir.SyncInfo(
                        on_wait=rest, on_update=list(si.on_update) if si.on_update else [])
                new.append(inst)
            bb.instructions = new


def _layernorm_tile(nc, w, x_t, g_bc, b_bc, eps_t, out_r, affine=True):
    """LayerNorm of one [128, DIM] fp32 tile -> bf16 tile (token-major)."""
    BNF = nc.vector.BN_STATS_FMAX
    nsub = DIM // BNF
    stats = w.tile([P, nsub, nc.vector.BN_STATS_DIM], f32, tag="ln_stats")
    xg = x_t[:].rearrange("p (s f) -> p s f", f=BNF)
    for s_ in range(nsub):
        nc.vector.bn_stats(out=stats[:, s_, :], in_=xg[:, s_, :])
    mv = w.tile([P, nc.vector.BN_AGGR_DIM], f32, tag="ln_mv")
    nc.vector.bn_aggr(out=mv, in_=stats)
    rstd = w.tile([P, 1], f32, tag="ln_rstd")
    nc.scalar.activation(out=rstd, in_=mv[:, 1:2], func=AF.Sqrt, bias=eps_t, scale=1.0)
    nc.vector.reciprocal(out=rstd, in_=rstd)
    if affine:
        tmp = w.tile([P, DIM], f32, tag="ln_tmp")
        nc.vector.tensor_scalar(out=tmp, in0=x_t, scalar1=mv[:, 0:1], scalar2=rstd,
                                op0=ALU.subtract, op1=ALU.mult)
        nc.vector.tensor_mul(out=tmp, in0=tmp, in1=g_bc)
        nc.vector.tensor_add(out=out_r, in0=tmp, in1=b_bc)
    else:
        nc.vector.tensor_scalar(out=out_r, in0=x_t, scalar1=mv[:, 0:1], scalar2=rstd,
                                op0=ALU.subtract, op1=ALU.mult)


def build_kernel(ln1_affine=True, ln2_affine=True, b2fin_zero=False, pairs=True,
                 fp8_proj=False):
    nc = bass.Bass(num_devices=N_CORES)
    groups = REPLICA_PAIRS if pairs else [[c] for c in range(N_CORES)]
    gsz = 2 if pairs else 1
    wdt = f8 if fp8_proj else bf16

    x_own = nc.dram_tensor("x_own", [T_OWN, DIM], f32, kind="ExternalInput")
    m_scale = nc.dram_tensor("m_scale", [1], f32, kind="ExternalInput")
    # [128, kt, d]: row (kt*128+p) of the original [DIM, DIM] weight
    wq_l = nc.dram_tensor("wq_l", [P, D_T, DIM], wdt, kind="ExternalInput")
    wk_l = nc.dram_tensor("wk_l", [P, D_T, DIM], wdt, kind="ExternalInput")
    wv_l = nc.dram_tensor("wv_l", [P, D_T, DIM], wdt, kind="ExternalInput")
    wo_l = nc.dram_tensor("wo_l", [P, D_T, DIM], bf16, kind="ExternalInput")
    ln1_g = nc.dram_tensor("ln1_g", [DIM], f32, kind="ExternalInput")
    ln1_b = nc.dram_tensor("ln1_b", [DIM], f32, kind="ExternalInput")
    ln2_g = nc.dram_tensor("ln2_g", [DIM], f32, kind="ExternalInput")
    ln2_b = nc.dram_tensor("ln2_b", [DIM], f32, kind="ExternalInput")
    # [lvl, 128, half, kt, 2048]: w1[lvl, kt*128+p, half*2048+c]
    w1_l = nc.dram_tensor("w1_l", [LEVELS, P, 2, D_T, HID // 2], f8, kind="ExternalInput")
    # [lvl, 128, half, kt16, 1024]: w2[lvl, half*2048+kt*128+p, d]
    w2_l = nc.dram_tensor("w2_l", [LEVELS, P, 2, H_T // 2, DIM], f8, kind="ExternalInput")
    b1_l = nc.dram_tensor("b1_l", [LEVELS, P, H_T], f32, kind="ExternalInput")
    b2_l = nc.dram_tensor("b2_l", [LEVELS - 1, P, D_T], f32, kind="ExternalInput")
    b2fin = nc.dram_tensor("b2fin", [DIM], f32, kind="ExternalInput")
    maskT = nc.dram_tensor("maskT", [CHUNK, CHUNK], f32, kind="ExternalInput")
    out = nc.dram_tensor("out", [T_OWN, DIM], f32, kind="ExternalOutput")

    with tile.TileContext(nc) as tc, ExitStack() as top:
        consts = top.enter_context(tc.tile_pool(name="consts", bufs=1))
        ident_f = consts.tile([P, P], f32)
        make_identity(nc, ident_f)
        ident = consts.tile([P, P], bf16)
        nc.vector.tensor_copy(out=ident, in_=ident_f)
        eps_t = consts.tile([P, 1], f32)
        nc.vector.memset(eps_t, EPS)
        # combined super-chunk score mask [e, S0(c0)|S0(c1)|S1]:
        # causal for (k0,q0), all-1 for (k0,q1), causal for (k1,q1)
        maskC = consts.tile([CHUNK, 3 * CHUNK], f32)
        nc.sync.dma_start(out=maskC[:, 0:CHUNK], in_=maskT.ap())
        nc.vector.memset(maskC[:, CHUNK:2 * CHUNK], 1.0)
        nc.sync.dma_start(out=maskC[:, 2 * CHUNK:3 * CHUNK], in_=maskT.ap())
        g1 = b1 = g2 = b2 = None
        if ln1_affine:
            g1 = consts.tile([P, DIM], bf16)
            b1 = consts.tile([P, DIM], bf16)
            nc.sync.dma_start(out=g1, in_=ln1_g.ap()[None, :].partition_broadcast(P).opt())
            nc.sync.dma_start(out=b1, in_=ln1_b.ap()[None, :].partition_broadcast(P).opt())
        if ln2_affine:
            g2 = consts.tile([P, DIM], bf16)
            b2 = consts.tile([P, DIM], bf16)
            nc.sync.dma_start(out=g2, in_=ln2_g.ap()[None, :].partition_broadcast(P).opt())
            nc.sync.dma_start(out=b2, in_=ln2_b.ap()[None, :].partition_broadcast(P).opt())
        msc = consts.tile([P, 1], f32)
        nc.sync.dma_start(out=msc, in_=m_scale.ap()[None, :].partition_broadcast(P).opt())

        # persistent across phases: FFN input h2^T (feature-major, fp8)
        persist = top.enter_context(tc.tile_pool(name="persist", bufs=1))
        hT = persist.tile([P, D_T, T_OWN], f8)
        dram = top.enter_context(tc.tile_pool(name="dram", bufs=1, space="DRAM"))
        x2_d = dram.tile([N_OWN_T, P, DIM], bf16)
        mt_d = dram.tile([P, D_T * HD], bf16)
        ag_d = dram.tile([gsz, P, D_T * HD], bf16)

        # qT / yT persist until the post-collective fixup + wo projection
        mid = ExitStack()
        qyp = mid.enter_context(tc.tile_pool(name="qyp", bufs=1))
        qT_p = qyp.tile([P, D_T, T_OWN], bf16)
        yT_p = qyp.tile([P, D_T, T_OWN], bf16)
        wo_pool = mid.enter_context(tc.tile_pool(name="wo_pool", bufs=1))
        wo_s = wo_pool.tile([P, D_T, DIM], bf16)
        nc.sync.dma_start(out=wo_s, in_=wo_l.ap())

        # ---------------- attention ----------------
        ab = ExitStack()
        wqkv = ab.enter_context(tc.tile_pool(name="wqkv", bufs=1))
        wq_s = wqkv.tile([P, D_T, DIM], wdt)
        wk_s = wqkv.tile([P, D_T, DIM], wdt)
        wv_s = wqkv.tile([P, D_T, DIM], wdt)
        nc.sync.dma_start(out=wq_s, in_=wq_l.ap())
        nc.sync.dma_start(out=wk_s, in_=wk_l.ap())
        nc.sync.dma_start(out=wv_s, in_=wv_l.ap())

        mt_pool = ab.enter_context(tc.tile_pool(name="mt", bufs=1))
        Mt_f = mt_pool.tile([P, D_T, HD], f32)   # partitions = d of head pair
        Mt_s = mt_pool.tile([P, D_T, HD], bf16)
        nc.vector.memset(Mt_f, 0.0)
        nc.vector.memset(Mt_s, 0.0)

        ln_w = ab.enter_context(tc.tile_pool(name="ln_w", bufs=2))
        xp = ab.enter_context(tc.tile_pool(name="xp", bufs=1))
        hrp = ab.enter_context(tc.tile_pool(name="hrp", bufs=1))
        h1Tp = ab.enter_context(tc.tile_pool(name="h1Tp", bufs=2))
        kvp = ab.enter_context(tc.tile_pool(name="kvp", bufs=2))
        vp = ab.enter_context(tc.tile_pool(name="vp", bufs=1))
        scw = ab.enter_context(tc.tile_pool(name="scw", bufs=3))
        # PSUM budget (8 banks): tp 2 (transposes incl. kc) + mm 4 (proj +
        # scores + fixup, all [P,512] f32 sharing one tag) + ymt 2
        ps_tp = ab.enter_context(tc.tile_pool(name="ps_tp", bufs=2, space="PSUM"))
        ps_mm = ab.enter_context(tc.tile_pool(name="ps_mm", bufs=4, space="PSUM"))
        ps_ymt = ab.enter_context(tc.tile_pool(name="ps_ymt", bufs=2, space="PSUM"))

        for blk in range(N_BLK):
            tok0 = blk * BLK
            # LayerNorm1 + transpose into feature-major h1T
            h1T = h1Tp.tile([P, D_T, BLK], wdt, tag="h1T")
            for t in range(NTT):
                x_t = xp.tile([P, DIM], f32, tag="x")
                nc.sync.dma_start(out=x_t, in_=x_own.ap()[tok0 + t * P:tok0 + (t + 1) * P, :])
                h_r = hrp.tile([P, DIM], bf16, tag="h1r")
                _layernorm_tile(nc, ln_w, x_t, g1, b1, eps_t, h_r, affine=ln1_affine)
                for fp in range(D_T // 2):
                    tps = ps_tp.tile([P, 2, P], bf16, tag="tp")
                    for j in range(2):
                        fi = 2 * fp + j
                        nc.tensor.transpose(tps[:, j, :], h_r[:, fi * P:(fi + 1) * P], ident)
                    nc.vector.tensor_copy(out=h1T[:, 2 * fp:2 * fp + 2, t * P:(t + 1) * P],
                                          in_=tps)
            # q, k: feature-major [feat, tok]
            kT = kvp.tile([P, D_T, BLK], bf16, tag="kT")
            for (w_s, dst, off) in ((wq_s, qT_p, tok0), (wk_s, kT, 0)):
                for m in range(D_T):
                    pst = ps_mm.tile([P, BLK], f32, tag="pst")
                    if fp8_proj:
                        for kk in range(D_T // 2):
                            nc.tensor.matmul(
                                pst, w_s[:, 2 * kk:2 * kk + 2, m * P:(m + 1) * P],
                                h1T[:, 2 * kk:2 * kk + 2, :],
                                start=(kk == 0), stop=(kk == D_T // 2 - 1),
                                perf_mode=PM.DoubleRow)
                        nc.scalar.activation(out=dst[:, m, off:off + BLK], in_=pst,
                                             func=AF.Identity, bias=0.0, scale=INV_W8)
                    else:
                        for k in range(D_T):
                            nc.tensor.matmul(pst, w_s[:, k, m * P:(m + 1) * P], h1T[:, k, :],
                                             start=(k == 0), stop=(k == D_T - 1))
                        nc.scalar.copy(out=dst[:, m, off:off + BLK], in_=pst)
            # v: token-major [tok, feat]
            v = vp.tile([P, NTT, DIM], bf16, tag="v")
            for ti in range(NTT):
                for nh in range(2):
                    pst = ps_mm.tile([P, 512], f32, tag="pst")
                    if fp8_proj:
                        for kk in range(D_T // 2):
                            nc.tensor.matmul(
                                pst, h1T[:, 2 * kk:2 * kk + 2, ti * P:(ti + 1) * P],
                                wv_s[:, 2 * kk:2 * kk + 2, nh * 512:(nh + 1) * 512],
                                start=(kk == 0), stop=(kk == D_T // 2 - 1),
                                perf_mode=PM.DoubleRow)
                        nc.scalar.activation(out=v[:, ti, nh * 512:(nh + 1) * 512], in_=pst,
                                             func=AF.Identity, bias=0.0, scale=INV_W8)
                    else:
                        for k in range(D_T):
                            nc.tensor.matmul(pst, h1T[:, k, ti * P:(ti + 1) * P],
                                             wv_s[:, k, nh * 512:(nh + 1) * 512],
                                             start=(k == 0), stop=(k == D_T - 1))
                        nc.scalar.copy(out=v[:, ti, nh * 512:(nh + 1) * 512], in_=pst)
            # scan: super-chunks of 256 tokens, head-pair packed, feature-major y
            for sch in range(NTT // 2):
                s0 = sch * 2 * P          # in-block offset of chunk c0
                q0 = tok0 + s0            # global offset
                for fi in range(D_T):
                    h0c = (2 * fi) * HD   # head col offsets in token-major v
                    h1c = (2 * fi + 1) * HD
                    # k chunk transposes (both heads at once: [tok, d-pair])
                    kc_ps = ps_tp.tile([P, 2, P], bf16, tag="tp")
                    nc.tensor.transpose(kc_ps[:, 0, :], kT[:, fi, s0:s0 + P], ident)
                    nc.tensor.transpose(kc_ps[:, 1, :], kT[:, fi, s0 + P:s0 + 2 * P], ident)
                    kc = scw.tile([P, 2, P], bf16, tag="kc")
                    nc.scalar.copy(out=kc, in_=kc_ps)
                    # scores [S0(256)|S1(128)|pad] per head (K=64 -> row-group
                    # packed pair), one [P,512] bank each
                    s_ps = [ps_mm.tile([P, 512], f32, tag="pst",
                                       name=f"s_{blk}_{sch}_{fi}_{hh}")
                            for hh in range(2)]
                    for hh in range(2):
                        pb = hh * HD
                        nc.tensor.matmul(s_ps[hh][:, 0:2 * P],
                                         kT[pb:pb + HD, fi, s0:s0 + P],
                                         qT_p[pb:pb + HD, fi, q0:q0 + 2 * P],
                                         start=True, stop=True)
                        nc.tensor.matmul(s_ps[hh][:, 2 * P:3 * P],
                                         kT[pb:pb + HD, fi, s0 + P:s0 + 2 * P],
                                         qT_p[pb:pb + HD, fi, q0 + P:q0 + 2 * P],
                                         start=True, stop=True)
                    s_r = scw.tile([P, 2, 3 * P], bf16, tag="s_r")
                    nc.vector.tensor_mul(out=s_r[:, 0, :], in0=s_ps[0][:, 0:3 * P], in1=maskC)
                    nc.vector.tensor_mul(out=s_r[:, 1, :], in0=s_ps[1][:, 0:3 * P], in1=maskC)
                    # y^T accumulation (intra col-packed + memory diag-packed)
                    # and M update (col-packed), sharing one PSUM bank
                    ymt = ps_ymt.tile([P, 2 * P + HD], f32, tag="ymt")
                    y_ps = ymt[:, 0:2 * P]
                    mt_ps = ymt[:, 2 * P:2 * P + HD]
                    nc.tensor.matmul(y_ps[0:HD, :], v[:, sch * 2, h0c:h0c + HD],
                                     s_r[:, 0, 0:2 * P], start=True, stop=False)
                    nc.tensor.matmul(y_ps[HD:P, :], v[:, sch * 2, h1c:h1c + HD],
                                     s_r[:, 1, 0:2 * P], start=True, stop=False)
                    nc.tensor.matmul(y_ps[0:HD, P:2 * P], v[:, sch * 2 + 1, h0c:h0c + HD],
                                     s_r[:, 0, 2 * P:3 * P], start=False, stop=False)
                    nc.tensor.matmul(y_ps[HD:P, P:2 * P], v[:, sch * 2 + 1, h1c:h1c + HD],
                                     s_r[:, 1, 2 * P:3 * P], start=False, stop=False)
                    nc.tensor.matmul(y_ps[0:HD, :], Mt_s[0:HD, fi, :],
                                     qT_p[0:HD, fi, q0:q0 + 2 * P], start=False, stop=True)
                    nc.tensor.matmul(y_ps[HD:P, :], Mt_s[HD:P, fi, :],
                                     qT_p[HD:P, fi, q0:q0 + 2 * P], start=False, stop=True)
                    nc.scalar.copy(out=yT_p[:, fi, q0:q0 + 2 * P], in_=y_ps)
                    # M update (col-packed pair per chunk)
                    nc.tensor.matmul(mt_ps[0:HD, :], kc[:, 0, 0:HD],
                                     v[:, sch * 2, h0c:h0c + HD], start=True, stop=False)
                    nc.tensor.matmul(mt_ps[HD:P, :], kc[:, 0, HD:P],
                                     v[:, sch * 2, h1c:h1c + HD], start=True, stop=False)
                    nc.tensor.matmul(mt_ps[0:HD, :], kc[:, 1, 0:HD],
                                     v[:, sch * 2 + 1, h0c:h0c + HD], start=False, stop=True)
                    nc.tensor.matmul(mt_ps[HD:P, :], kc[:, 1, HD:P],
                                     v[:, sch * 2 + 1, h1c:h1c + HD], start=False, stop=True)
                    nc.vector.tensor_add(out=Mt_f[:, fi, :], in0=Mt_f[:, fi, :], in1=mt_ps)
                    nc.vector.tensor_copy(out=Mt_s[:, fi, :], in_=Mt_f[:, fi, :])

        # ---------------- pairwise memory-state exchange (bf16) ----------------
        nc.gpsimd.dma_start(mt_d[:], Mt_s[:].rearrange("p a b -> p (a b)"))
        nc.gpsimd.collective_compute(
            "AllGather",
            mybir.AluOpType.bypass,
            replica_groups=groups,
            ins=[mt_d.opt()],
            outs=[ag_d.opt()],
        )
        mrem = mt_pool.tile([P, D_T, HD], bf16)
        nc.gpsimd.dma_start(mrem[:].rearrange("p a b -> p (a b)"), ag_d[0])
        # even cores start the sequence: scale their received M to zero
        nc.vector.tensor_scalar_mul(out=mrem, in0=mrem, scalar1=msc[:, 0:1])
        # y += q @ M_remote over all own tokens (group-outer so the wo pass
        # for the first token tiles can start after 1/4 of the fixup)
        for g in range(T_OWN // 512):
            for fi in range(D_T):
                ps = ps_mm.tile([P, 512], f32, tag="pst")
                nc.tensor.matmul(ps[0:HD, :], mrem[0:HD, fi, :],
                                 qT_p[0:HD, fi, g * 512:(g + 1) * 512],
                                 start=True, stop=True)
                nc.tensor.matmul(ps[HD:P, :], mrem[HD:P, fi, :],
                                 qT_p[HD:P, fi, g * 512:(g + 1) * 512],
                                 start=True, stop=True)
                nc.vector.tensor_add(out=yT_p[:, fi, g * 512:(g + 1) * 512],
                                     in0=yT_p[:, fi, g * 512:(g + 1) * 512], in1=ps)

        ab.close()

        # ---------------- epilogue: wo, residual, LN2, h2^T ----------------
        ep = ExitStack()
        epw = ep.enter_context(tc.tile_pool(name="epw", bufs=3))
        ep1 = ep.enter_context(tc.tile_pool(name="ep1", bufs=1))
        ln_w2 = ep.enter_context(tc.tile_pool(name="ln_w2", bufs=3))
        ps_wo = ep.enter_context(tc.tile_pool(name="ps_wo", bufs=4, space="PSUM"))
        ps_t2 = ep.enter_context(tc.tile_pool(name="ps_t2", bufs=4, space="PSUM"))
        b2bc = None
        if not b2fin_zero:
            b2bc = ep1.tile([P, DIM], f32)
            nc.sync.dma_start(out=b2bc, in_=b2fin.ap()[None, :].partition_broadcast(P).opt())
        for ti in range(N_OWN_T):
            x_t = epw.tile([P, DIM], f32, tag="ex")
            nc.sync.dma_start(out=x_t, in_=x_own.ap()[ti * P:(ti + 1) * P, :])
            x2f = epw.tile([P, DIM], f32, tag="x2f")
            for nh in range(2):
                pst = ps_wo.tile([P, 512], f32, tag="wo")
                for k in range(D_T):
                    nc.tensor.matmul(pst, yT_p[:, k, ti * P:(ti + 1) * P],
                                     wo_s[:, k, nh * 512:(nh + 1) * 512],
                                     start=(k == 0), stop=(k == D_T - 1))
                nc.vector.tensor_add(out=x2f[:, nh * 512:(nh + 1) * 512],
                                     in0=x_t[:, nh * 512:(nh + 1) * 512], in1=pst)
            # spill x2 (+ final-level FFN bias, pre-added) for the output residual
            x2b = epw.tile([P, DIM], bf16, tag="x2b")
            if b2fin_zero:
                nc.scalar.copy(out=x2b, in_=x2f)
            else:
                nc.vector.tensor_add(out=x2b, in0=x2f, in1=b2bc)
            nc.scalar.dma_start(out=x2_d[ti], in_=x2b)
            h2_r = epw.tile([P, DIM], bf16, tag="h2r")
            _layernorm_tile(nc, ln_w2, x2f, g2, b2, eps_t, h2_r, affine=ln2_affine)
            for fi in range(D_T):
                tps = ps_t2.tile([P, P], bf16, tag="tp2")
                nc.tensor.transpose(tps, h2_r[:, fi * P:(fi + 1) * P], ident)
                nc.scalar.copy(out=hT[:, fi, ti * P:(ti + 1) * P], in_=tps)
        ep.close()
        mid.close()

        # ---------------- CMS FFN ----------------
        n_tt = T_OWN // TT
        with ExitStack() as ffn:
            w1s = ffn.enter_context(tc.tile_pool(name="w1s", bufs=1))
            w2s = ffn.enter_context(tc.tile_pool(name="w2s", bufs=1))
            bp = ffn.enter_context(tc.tile_pool(name="ffn_b", bufs=2))
            big = ffn.enter_context(tc.tile_pool(name="ffn_big", bufs=1))
            upg = big.tile([P, H_T, T_OWN], f8)     # gelu acts, full hidden
            ow = ffn.enter_context(tc.tile_pool(name="ow", bufs=3))
            ps_up = ffn.enter_context(tc.tile_pool(name="ps_up", bufs=4, space="PSUM"))
            ps_dn = ffn.enter_context(tc.tile_pool(name="ps_dn", bufs=4, space="PSUM"))

            def load_level(lvl):
                w1_t = w1s.tile([P, 2, D_T, HID // 2], f8, tag="w1t")
                for half in range(2):
                    nc.sync.dma_start(out=w1_t[:, half], in_=w1_l.ap()[lvl, :, half])
                w2_t = w2s.tile([P, H_T, DIM], f8, tag="w2t")
                for half in range(2):
                    nc.sync.dma_start(
                        out=w2_t[:, half * (H_T // 2):(half + 1) * (H_T // 2), :],
                        in_=w2_l.ap()[lvl, :, half])
                b1_t = bp.tile([P, H_T], f32, tag="b1")
                nc.sync.dma_start(out=b1_t, in_=b1_l.ap()[lvl])
                b2_t = None
                if lvl < LEVELS - 1:
                    b2_t = bp.tile([P, D_T], f32, tag="b2")
                    nc.sync.dma_start(out=b2_t, in_=b2_l.ap()[lvl])
                return w1_t, w2_t, b1_t, b2_t

            for lvl in range(LEVELS):
                w1_t, w2_t, b1_t, b2_t = load_level(lvl)
                # up: h @ w1 -> gelu (feature-major hidden)
                for mh in range(H_T):
                    half, ml = mh // (H_T // 2), mh % (H_T // 2)
                    w1_lhs = w1_t[:, half]
                    for tt in range(n_tt):
                        psl = ps_up.tile([P, TT], f32, tag="up", name=f"up_{lvl}_{mh}_{tt}")
                        for kk in range(D_T // 2):
                            nc.tensor.matmul(
                                psl, w1_lhs[:, 2 * kk:2 * kk + 2, ml * P:(ml + 1) * P],
                                hT[:, 2 * kk:2 * kk + 2, tt * TT:(tt + 1) * TT],
                                start=(kk == 0), stop=(kk == D_T // 2 - 1),
                                perf_mode=PM.DoubleRow)
                        nc.scalar.activation(
                            out=upg[:, mh, tt * TT:(tt + 1) * TT], in_=psl,
                            func=AF.Gelu_apprx_tanh,
                            bias=b1_t[:, mh:mh + 1], scale=INV_W8)
                if lvl < LEVELS - 1:
                    # down, feature-major back into hT (full-hidden PSUM group)
                    for md in range(D_T):
                        for tt in range(n_tt):
                            psl = ps_dn.tile([P, TT], f32, tag="dn", name=f"dn_{lvl}_{md}_{tt}")
                            for kk in range(H_T // 2):
                                nc.tensor.matmul(
                                    psl, w2_t[:, 2 * kk:2 * kk + 2, md * P:(md + 1) * P],
                                    upg[:, 2 * kk:2 * kk + 2, tt * TT:(tt + 1) * TT],
                                    start=(kk == 0), stop=(kk == H_T // 2 - 1),
                                    perf_mode=PM.DoubleRow)
                            nc.scalar.activation(
                                out=hT[:, md, tt * TT:(tt + 1) * TT], in_=psl,
                                func=AF.Identity, bias=b2_t[:, md:md + 1], scale=INV_W8)
                else:
                    # last level: token-major output, fused residual + store
                    for ti in range(N_OWN_T):
                        x2_t = ow.tile([P, DIM], bf16, tag="ox2")
                        nc.sync.dma_start(out=x2_t, in_=x2_d[ti])
                        o_t = ow.tile([P, DIM], f32, tag="oo")
                        for nh in range(2):
                            psl = ps_dn.tile([P, 512], f32, tag="dn", name=f"fin_{ti}_{nh}")
                            for kk in range(H_T // 2):
                                nc.tensor.matmul(
                                    psl, upg[:, 2 * kk:2 * kk + 2, ti * P:(ti + 1) * P],
                                    w2_t[:, 2 * kk:2 * kk + 2, nh * 512:(nh + 1) * 512],
                                    start=(kk == 0), stop=(kk == H_T // 2 - 1),
                                    perf_mode=PM.DoubleRow)
                            htmp = ow.tile([P, 512], bf16, tag="oh")
                            nc.scalar.activation(out=htmp, in_=psl, func=AF.Identity,
                                                 bias=0.0, scale=INV_W8)
                            nc.vector.tensor_add(out=o_t[:, nh * 512:(nh + 1) * 512],
                                                 in0=x2_t[:, nh * 512:(nh + 1) * 512],
                                                 in1=htmp)
                        nc.scalar.dma_start(out=out.ap()[ti * P:(ti + 1) * P, :], in_=o_t)

    _split_multi_waits(nc)
    return nc


_NC_CACHE = {}
LAST_RESULT = None


def _get_nc(key, **kw):
    if key not in _NC_CACHE:
        _NC_CACHE[key] = build_kernel(**kw)
    return _NC_CACHE[key]


def kernel(x, ln1_g, ln1_b, wq, wk, wv, wo, ln2_g, ln2_b,
           cms_w1, cms_b1, cms_w2, cms_b2, **extra):
    import ml_dtypes
    bf = ml_dtypes.bfloat16
    f8h = ml_dtypes.float8_e4m3
    x = np.asarray(x, np.float32)
    maskT = np.triu(np.ones((CHUNK, CHUNK), np.float32))  # maskT[e,c] = e<=c

    FP8_PROJ = False

    def wlin(w, f8w=False):  # [DIM, DIM] -> [128, kt, DIM]
        a = np.asarray(w, np.float32).reshape(D_T, P, DIM).transpose(1, 0, 2)
        if f8w:
            return np.ascontiguousarray((a * W8_SCALE).astype(f8h))
        return np.ascontiguousarray(a.astype(bf))

    w1s = (np.asarray(cms_w1, np.float32) * W8_SCALE)
    w1_h = np.ascontiguousarray(
        w1s.reshape(LEVELS, D_T, P, 2, HID // 2).transpose(0, 2, 3, 1, 4).astype(f8h))
    w2s = (np.asarray(cms_w2, np.float32) * W8_SCALE)
    w2_h = np.ascontiguousarray(
        w2s.reshape(LEVELS, 2, H_T // 2, P, DIM).transpose(0, 3, 1, 2, 4).astype(f8h))
    b1_h = np.ascontiguousarray(
        np.asarray(cms_b1, np.float32).reshape(LEVELS, H_T, P).transpose(0, 2, 1))
    b2a = np.asarray(cms_b2, np.float32)
    b2_h = np.ascontiguousarray(b2a[:LEVELS - 1].reshape(LEVELS - 1, D_T, P).transpose(0, 2, 1))
    b2fin = np.ascontiguousarray(b2a[LEVELS - 1])

    ln1_g = np.asarray(ln1_g, np.float32)
    ln1_b = np.asarray(ln1_b, np.float32)
    ln2_g = np.asarray(ln2_g, np.float32)
    ln2_b = np.asarray(ln2_b, np.float32)
    ln1_affine = not (np.all(ln1_g == 1.0) and np.all(ln1_b == 0.0))
    ln2_affine = not (np.all(ln2_g == 1.0) and np.all(ln2_b == 0.0))
    b2fin_zero = bool(np.all(b2fin == 0.0))

    common = {
        "wq_l": wlin(wq, FP8_PROJ), "wk_l": wlin(wk, FP8_PROJ),
        "wv_l": wlin(wv, FP8_PROJ), "wo_l": wlin(wo),
        "ln1_g": ln1_g, "ln1_b": ln1_b, "ln2_g": ln2_g, "ln2_b": ln2_b,
        "w1_l": w1_h, "w2_l": w2_h, "b1_l": b1_h, "b2_l": b2_h, "b2fin": b2fin,
        "maskT": maskT,
    }
    in_maps = []
    for c in range(N_CORES):
        b, half = c // 2, c % 2
        own = x[b, half * T_OWN:(half + 1) * T_OWN]
        in_maps.append({**common, "x_own": np.ascontiguousarray(own),
                        "m_scale": np.array([float(half)], np.float32)})
    nc = _get_nc(("v3", ln1_affine, ln2_affine, b2fin_zero, FP8_PROJ),
                 ln1_affine=ln1_affine, ln2_affine=ln2_affine, b2fin_zero=b2fin_zero,
                 fp8_proj=FP8_PROJ)
    res = run_bass_kernel_spmd(nc, in_maps, core_ids=list(range(N_CORES)))
    global LAST_RESULT
    LAST_RESULT = res
    out = np.empty((B, S, DIM), np.float32)
    for c in range(N_CORES):
        b, half = c // 2, c % 2
        out[b, half * T_OWN:(half + 1) * T_OWN] = res.results[c]["out"]
    return out
